# revision 3
# baseline (speedup 1.0000x reference)
"""Trainium2 Bass kernel for nn_Bidir_Attention (top-k masked bidirectional
cross-attention).

Data-parallel over batch: each of the 8 NeuronCores processes one batch
element end-to-end (QKV GEMM, scores, softmax, exact top-16 mask via
max8+match_replace, masked AV). W_qkv is replicated.

Precision strategy: the top-16 selection must reproduce the reference's
exact-fp32 S ranking (selection swaps cost ~5e-2 rel err), so the Q/K GEMMs
and the S=QK^T matmul stay in fp32 (4 cyc/row on the PE). The V GEMM, the
A^T transposes and the AV matmul only affect output VALUES, so they run in
float32r (1 cyc/row) - ~1e-4 rel noise, far under the 2e-2 gate.

Self-contained: hardcodes B=8, N=2048, D=1024, topk=16.
"""

import sys

import numpy as np

for _p in ("/opt/trn_rl_repo", "/root/.axon_site/_ro/trn_rl_repo"):
    if _p not in sys.path:
        sys.path.append(_p)

import concourse.bacc as bacc
import concourse.mybir as mybir
from concourse.tile import TileContext
from concourse.masks import make_identity
from concourse.bass_utils import run_bass_kernel_spmd

B = 8
N = 2048
D = 1024
NT = N // 128          # 16 row tiles
DT = D // 128          # 8 contraction tiles
TOPK = 16
SCALE = float(1.0 / np.sqrt(D))
NEG = -1e30
F32 = mybir.dt.float32
F32R = mybir.dt.float32r


def _phase_a(nc, pools, x_dram, ident_f, wqk, wv, qt_dram, kt_dram, v_dram):
    """QKV GEMM for one feature: writes Q^T (pre-scaled), K^T (both [D,N],
    fp32) and V ([N,D], f32r) to DRAM scratch. x is consumed transposed via
    PE; x^T is kept in fp32 (for exact Q/K) and cast to f32r (for V)."""
    sb, ps = pools
    for j in range(4):                      # supertiles of 512 rows
        xs = []
        for nsub in range(4):
            x = sb.tile([128, D], F32, tag=f"x{nsub}", bufs=2)
            nc.sync.dma_start(
                out=x[:], in_=x_dram.ap()[j * 512 + nsub * 128: j * 512 + (nsub + 1) * 128, :])
            xs.append(x)
        xT = sb.tile([128, DT, 512], F32, tag="xT", bufs=1)
        xTr = sb.tile([128, DT, 512], F32R, tag="xTr", bufs=1)
        for nsub in range(4):
            for di in range(DT):
                tp = ps.tile([128, 128], F32, tag="tp")
                nc.tensor.transpose(tp[:], xs[nsub][:, di * 128:(di + 1) * 128], ident_f[:])
                sl = (di, slice(nsub * 128, (nsub + 1) * 128))
                if (nsub * DT + di) % 2:
                    nc.vector.tensor_copy(xT[:, sl[0], sl[1]], tp[:])
                    nc.scalar.copy(xTr[:, sl[0], sl[1]], tp[:])
                else:
                    nc.scalar.copy(xT[:, sl[0], sl[1]], tp[:])
                    nc.vector.tensor_copy(xTr[:, sl[0], sl[1]], tp[:])
        # Q^T and K^T: [dout 128-tile, n 512] pieces (fp32 exact)
        for t in range(16):
            qk_ps = ps.tile([128, 512], F32, tag="qk_ps")
            for di in range(DT):
                nc.tensor.matmul(qk_ps[:], wqk[di][t][:], xT[:, di, :],
                                 start=(di == 0), stop=(di == DT - 1))
            o = sb.tile([128, 512], F32, tag="qko")
            if t < 8:
                nc.scalar.mul(o[:], qk_ps[:], SCALE)   # fold in 1/sqrt(D)
                dst = qt_dram
                r0 = t * 128
            else:
                nc.vector.tensor_copy(o[:], qk_ps[:])
                dst = kt_dram
                r0 = (t - 8) * 128
            nc.gpsimd.dma_start(
                out=dst.ap()[r0:r0 + 128, j * 512:(j + 1) * 512], in_=o[:])
        # V: natural layout [n 128-tile, dout 512] pieces, f32r single-pass
        for nsub in range(4):
            for c in range(2):
                v_ps = ps.tile([128, 512], F32, tag="v_ps")
                nsl = slice(nsub * 128, (nsub + 1) * 128)
                for di in range(DT):
                    nc.tensor.matmul(v_ps[:], xTr[:, di, nsl],
                                     wv[di][:, c * 512:(c + 1) * 512],
                                     start=(di == 0), stop=(di == DT - 1))
                vo = sb.tile([128, 512], F32R, tag="vo")
                if (nsub * 2 + c) % 2:
                    nc.vector.tensor_copy(vo[:], v_ps[:])
                else:
                    nc.scalar.copy(vo[:], v_ps[:])
                nc.gpsimd.dma_start(
                    out=v_dram.ap()[j * 512 + nsub * 128: j * 512 + (nsub + 1) * 128,
                                    c * 512:(c + 1) * 512],
                    in_=vo[:])


def _phase_b(nc, pools, ident_f, ident_r, qt_dram, kt_dram, v_dram, out_dram):
    """One attention direction: S = Q^T.T @ K^T (fp32, pre-scaled), softmax
    row stats, exact top-16 mask via 2x(max8+match_replace), masked AV in
    f32r, 1/Z renormalization."""
    sbr, sb, ps = pools
    # residents
    kt = []
    for di in range(DT):
        t = sbr.tile([128, N], F32, tag=f"kt{di}", name=f"kt{di}")
        nc.sync.dma_start(out=t[:], in_=kt_dram.ap()[di * 128:(di + 1) * 128, :])
        kt.append(t)
    vres = []
    for nt in range(NT):
        t = sbr.tile([128, D], F32R, tag=f"v{nt}", name=f"v{nt}")
        nc.sync.dma_start(out=t[:], in_=v_dram.ap()[nt * 128:(nt + 1) * 128, :])
        vres.append(t)

    for qi in range(NT):
        qts = []
        for di in range(DT):
            t = sb.tile([128, 128], F32, tag=f"qt{di}", name=f"qt{di}")
            nc.sync.dma_start(
                out=t[:], in_=qt_dram.ap()[di * 128:(di + 1) * 128, qi * 128:(qi + 1) * 128])
            qts.append(t)
        ssb = sb.tile([128, N], F32, tag="ssb")
        for half in range(2):
            s_ps = ps.tile([128, N // 2], F32, tag="s_ps", bufs=2)
            for di in range(DT):
                for c in range(2):
                    nc.tensor.matmul(s_ps[:, c * 512:(c + 1) * 512], qts[di][:],
                                     kt[di][:, half * 1024 + c * 512:
                                            half * 1024 + (c + 1) * 512],
                                     start=(di == 0), stop=(di == DT - 1))
            nc.vector.tensor_copy(ssb[:, half * 1024:(half + 1) * 1024], s_ps[:])

        m0 = sb.tile([128, 8], F32, tag="m0")
        nc.vector.max(out=m0[:], in_=ssb[:])
        nm = sb.tile([128, 1], F32, tag="nm")
        nc.vector.tensor_scalar_mul(nm[:], m0[:, 0:1], -1.0)
        p = sb.tile([128, N], F32, tag="p")
        z = sb.tile([128, 1], F32, tag="z")
        nc.scalar.activation(p[:], ssb[:], mybir.ActivationFunctionType.Exp,
                             bias=nm[:], scale=1.0, accum_out=z[:])
        iz = sb.tile([128, 1], F32, tag="iz")
        nc.vector.reciprocal(iz[:], z[:])
        # exact top-16: two rounds of max8 + match_replace (in place on ssb,
        # which the Exp above has already consumed)
        nc.vector.match_replace(out=ssb[:], in_to_replace=m0[:], in_values=ssb[:],
                                imm_value=NEG)
        m8 = sb.tile([128, 8], F32, tag="m8")
        nc.vector.max(out=m8[:], in_=ssb[:])
        nc.vector.match_replace(out=ssb[:], in_to_replace=m8[:], in_values=ssb[:],
                                imm_value=NEG)
        # A = exp(S - m) where selected else 0, cast to f32r for the AV path
        pr = sb.tile([128, N], F32R, tag="pr", bufs=1)
        nc.vector.scalar_tensor_tensor(out=pr[:], in0=ssb[:], scalar=NEG, in1=p[:],
                                       op0=mybir.AluOpType.is_equal,
                                       op1=mybir.AluOpType.mult)
        # transpose A tiles (f32r, 1.5 cyc/row) for the AV matmul
        ats = []
        for kt_i in range(NT):
            tp = ps.tile([128, 128], F32R, tag="tp2")
            nc.tensor.transpose(tp[:], pr[:, kt_i * 128:(kt_i + 1) * 128], ident_r[:])
            at = sb.tile([128, 128], F32R, tag=f"at{kt_i}", name=f"at{kt_i}")
            if kt_i % 2:
                nc.vector.tensor_copy(at[:], tp[:])
            else:
                nc.scalar.copy(at[:], tp[:])
            ats.append(at)
        osb = sb.tile([128, D], F32, tag="osb")
        for h in range(2):
            o_ps = ps.tile([128, 512], F32, tag="o_ps")
            hs = slice(h * 512, (h + 1) * 512)
            for kt_i in range(NT):
                nc.tensor.matmul(o_ps[:], ats[kt_i][:], vres[kt_i][:, hs],
                                 start=(kt_i == 0), stop=(kt_i == NT - 1))
            nc.vector.tensor_scalar_mul(osb[:, hs], o_ps[:], iz[:])
        nc.gpsimd.dma_start(out=out_dram.ap()[qi * 128:(qi + 1) * 128, :], in_=osb[:])


def build():
    nc = bacc.Bacc()
    f1 = nc.declare_dram_parameter("feature1", [N, D], F32, isOutput=False)
    f2 = nc.declare_dram_parameter("feature2", [N, D], F32, isOutput=False)
    w = nc.declare_dram_parameter("w_qkv", [D, 3 * D], F32, isOutput=False)
    out1 = nc.declare_dram_parameter("out1", [N, D], F32, isOutput=True)
    out2 = nc.declare_dram_parameter("out2", [N, D], F32, isOutput=True)

    q1t = nc.dram_tensor("q1t", [D, N], F32)
    k1t = nc.dram_tensor("k1t", [D, N], F32)
    v1 = nc.dram_tensor("v1", [N, D], F32R)
    q2t = nc.dram_tensor("q2t", [D, N], F32)
    k2t = nc.dram_tensor("k2t", [D, N], F32)
    v2 = nc.dram_tensor("v2", [N, D], F32R)

    with TileContext(nc) as tc:
        with tc.tile_pool(name="const", bufs=1) as constp:
            ident_f = constp.tile([128, 128], F32, tag="id_f")
            make_identity(nc, ident_f[:])
            ident_r = constp.tile([128, 128], F32R, tag="id_r")
            nc.vector.tensor_copy(ident_r[:], ident_f[:])

            with (
                tc.tile_pool(name="wpool", bufs=1) as wp,
                tc.tile_pool(name="apool", bufs=1) as asb,
                tc.tile_pool(name="apsum", bufs=2, space="PSUM") as aps,
            ):
                # t-major load order: the first QK output group (t=0) needs
                # wqk[di][0] for all di, so those 8 tiles arrive first and the
                # first matmuls overlap the remaining W loads
                wqk = [[None] * 16 for _ in range(DT)]
                for t in range(16):
                    for di in range(DT):
                        wt = wp.tile([128, 128], F32, tag=f"w{di}_{t}",
                                     name=f"w{di}_{t}")
                        nc.sync.dma_start(
                            out=wt[:],
                            in_=w.ap()[di * 128:(di + 1) * 128, t * 128:(t + 1) * 128])
                        wqk[di][t] = wt
                wv = []
                for di in range(DT):
                    wt = wp.tile([128, 1024], F32R, tag=f"wvt{di}", name=f"wvt{di}")
                    # gpsimd DMA performs the (byte-identical) f32->f32r cast
                    nc.gpsimd.dma_start(
                        out=wt[:],
                        in_=w.ap()[di * 128:(di + 1) * 128, 2048:3072])
                    wv.append(wt)
                _phase_a(nc, (asb, aps), f1, ident_f, wqk, wv, q1t, k1t, v1)
                _phase_a(nc, (asb, aps), f2, ident_f, wqk, wv, q2t, k2t, v2)

            with (
                tc.tile_pool(name="bpool", bufs=1) as bsb,
                tc.tile_pool(name="bwork", bufs=2) as bwk,
                tc.tile_pool(name="bpsum", bufs=2, space="PSUM") as bps,
            ):
                _phase_b(nc, (bsb, bwk, bps), ident_f, ident_r, q1t, k2t, v2, out1)
                _phase_b(nc, (bsb, bwk, bps), ident_f, ident_r, q2t, k1t, v1, out2)
    return nc


_NC_CACHE = None


def _get_nc():
    global _NC_CACHE
    if _NC_CACHE is None:
        _NC_CACHE = build()
        _NC_CACHE.finalize()
    return _NC_CACHE


def kernel(feature1, feature2, W_qkv, topk):
    assert int(topk) == TOPK, f"kernel hardcodes topk=16, got {topk}"
    f1 = np.ascontiguousarray(np.asarray(feature1), dtype=np.float32)
    f2 = np.ascontiguousarray(np.asarray(feature2), dtype=np.float32)
    w = np.ascontiguousarray(np.asarray(W_qkv), dtype=np.float32)
    assert f1.shape == (B, N, D) and f2.shape == (B, N, D) and w.shape == (D, 3 * D)

    nc = _get_nc()
    in_maps = [{"feature1": f1[b], "feature2": f2[b], "w_qkv": w} for b in range(B)]
    try:
        res = run_bass_kernel_spmd(nc, in_maps, list(range(B))).results
    except Exception:
        # transient device faults have been observed; one retry on a fresh
        # execution usually clears them
        res = run_bass_kernel_spmd(nc, in_maps, list(range(B))).results
    o1 = np.stack([res[b]["out1"] for b in range(B)]).astype(np.float32)
    o2 = np.stack([res[b]["out2"] for b in range(B)]).astype(np.float32)
    return o1, o2


